# revision 3
# baseline (speedup 1.0000x reference)
"""Trainium2 Bass kernel for nn_BQNNModel (binary-quantum NN forward).

Reference computation (all fp32):
    h      = x @ fc1_w.T + fc1_b          # [B, H]
    h01    = clip((sign(h)+1)/2, 0, 1)    # {0, 0.5, 1}
    angle  = pi/2 + 0.5*(h01-0.5)*pi      # {pi/4, pi/2, 3pi/4}
    exp    = sin(angle) * sin(theta)[None]
    logits = exp @ fc_out_w.T + fc_out_b  # [B, C]

Sharding: pure data parallelism over batch across 8 cores (2048 rows each),
weights replicated.  No collectives needed (forward only).

Per-core device pipeline (hT layout — h is produced transposed so no on-chip
transposes are ever needed):
    xT  [1024, 2048] bf16 (host pre-cast + pre-transposed)
    hT[q-block] = sum_k W1T[k,q].T @ xT[k]        (TensorE, bf16, PSUM fp32)
    g   = (hT + b1[q]) > 0                        (VectorE, one tensor_scalar)
    sT  = Sin(g * (pi/2) + pi/4)                  (ScalarE activation, fp32)
    logitsT = sum_q W2T'[q].T @ sT[q]             (TensorE, fp32)
    outT = logitsT + b2                           (ScalarE Identity+bias)
where W2T'[q, c] = sin(theta_q) * fc_out_w[c, q] is folded on the host.

bf16 in matmul1 is safe: the output depends on h only through sign(h), and
sin(pi/4) == sin(3*pi/4), so even a sign flip of an |h|~0 element does not
change the result.
"""

import numpy as np
import ml_dtypes
from contextlib import ExitStack

B, F, H, C = 16384, 1024, 512, 10
NCORES = 8
R = B // NCORES          # 2048 rows per core
RC = 512                 # row chunk (matmul free dim)
P = 128
KB = F // P              # 8 contraction blocks
QB = H // P              # 4 hidden blocks
NCH = R // RC            # 4 row chunks per core

PI32 = np.float32(np.pi)
SIN_SCALE = float(np.float32(PI32 / np.float32(2.0)))   # pi/2 in fp32
SIN_BIAS = float(np.float32(PI32 / np.float32(4.0)))    # pi/4 in fp32

_CACHE = {}


def _build_program():
    import concourse.bass as bass  # noqa: F401
    import concourse.tile as tile
    from concourse import bacc, mybir

    nc = bacc.Bacc("TRN2", target_bir_lowering=False, debug=False,
                   num_devices=NCORES)

    xt = nc.dram_tensor("xt", [F, R], mybir.dt.bfloat16,
                        kind="ExternalInput").ap()
    w1t = nc.dram_tensor("w1t", [F, H], mybir.dt.bfloat16,
                         kind="ExternalInput").ap()
    b1 = nc.dram_tensor("b1", [H], mybir.dt.float32,
                        kind="ExternalInput").ap()
    w2t = nc.dram_tensor("w2t", [H, C], mybir.dt.float32,
                         kind="ExternalInput").ap()
    b2 = nc.dram_tensor("b2", [C, 1], mybir.dt.float32,
                        kind="ExternalInput").ap()
    outT = nc.dram_tensor("outT", [C, R], mybir.dt.float32,
                          kind="ExternalOutput").ap()

    with tile.TileContext(nc) as tc, ExitStack() as ctx:
        _kernel_body(ctx, tc, outT, xt, w1t, b1, w2t, b2, mybir)

    nc.compile()
    return nc


def _kernel_body(ctx, tc, outT, xt, w1t, b1, w2t, b2, mybir):
    nc = tc.nc

    consts = ctx.enter_context(tc.tile_pool(name="consts", bufs=1))
    xpool = ctx.enter_context(tc.tile_pool(name="xpool", bufs=2))
    gpool = ctx.enter_context(tc.tile_pool(name="gpool", bufs=3))
    spool = ctx.enter_context(tc.tile_pool(name="spool", bufs=2 * QB))
    opool = ctx.enter_context(tc.tile_pool(name="opool", bufs=1))
    psum1 = ctx.enter_context(tc.tile_pool(name="psum1", bufs=3, space="PSUM"))
    psum2 = ctx.enter_context(tc.tile_pool(name="psum2", bufs=2, space="PSUM"))

    xt_r = xt.rearrange("(ko p) r -> p ko r", p=P)

    # Replicated weights / biases.
    w1t_sb = consts.tile([P, KB, H], mybir.dt.bfloat16)
    nc.sync.dma_start(w1t_sb[:], w1t.rearrange("(ko p) h -> p ko h", p=P))
    w2t_sb = consts.tile([P, QB, C], mybir.dt.float32)
    nc.sync.dma_start(w2t_sb[:], w2t.rearrange("(qo p) c -> p qo c", p=P))
    b1_sb = consts.tile([P, QB], mybir.dt.float32)
    nc.sync.dma_start(b1_sb[:], b1.rearrange("(qo p) -> p qo", p=P))
    b2_sb = consts.tile([C, 1], mybir.dt.float32)
    nc.sync.dma_start(b2_sb[:], b2[:])
    sinb_sb = consts.tile([P, 1], mybir.dt.float32)
    nc.any.memset(sinb_sb[:], SIN_BIAS)

    out_sb = opool.tile([C, R], mybir.dt.float32)

    for c in range(NCH):
        x_sb = xpool.tile([P, KB, RC], mybir.dt.bfloat16)
        nc.sync.dma_start(x_sb[:], xt_r[:, :, c * RC:(c + 1) * RC])

        s_tiles = []
        for q in range(QB):
            hps = psum1.tile([P, RC], mybir.dt.float32)
            for k in range(KB):
                nc.tensor.matmul(
                    hps[:],
                    w1t_sb[:, k, q * P:(q + 1) * P],
                    x_sb[:, k, :],
                    start=(k == 0),
                    stop=(k == KB - 1),
                )
            # g = (h + b1) > 0  in {0.0, 1.0}; bf16 is exact for those.
            g = gpool.tile([P, RC], mybir.dt.bfloat16)
            nc.vector.tensor_scalar(
                out=g[:],
                in0=hps[:],
                scalar1=b1_sb[:, q:q + 1],
                scalar2=0.0,
                op0=mybir.AluOpType.add,
                op1=mybir.AluOpType.is_gt,
            )
            # sT = sin(g*pi/2 + pi/4)
            s = spool.tile([P, RC], mybir.dt.float32)
            nc.scalar.activation(
                s[:], g[:], mybir.ActivationFunctionType.Sin,
                bias=sinb_sb[:], scale=SIN_SCALE,
            )
            s_tiles.append(s)

        lps = psum2.tile([C, RC], mybir.dt.float32)
        for q in range(QB):
            nc.tensor.matmul(
                lps[:],
                w2t_sb[:, q, :],
                s_tiles[q][:],
                start=(q == 0),
                stop=(q == QB - 1),
            )
        # out = logits + b2 (per-partition bias), PSUM -> SBUF
        nc.scalar.activation(
            out_sb[:, c * RC:(c + 1) * RC], lps[:],
            mybir.ActivationFunctionType.Identity,
            bias=b2_sb[:], scale=1.0,
        )

    nc.sync.dma_start(outT[:], out_sb[:])


def _get_program():
    if "nc" not in _CACHE:
        _CACHE["nc"] = _build_program()
    return _CACHE["nc"]


def _prepare_in_maps(x, fc1_w, fc1_b, theta_quantum, fc_out_w, fc_out_b):
    bf16 = ml_dtypes.bfloat16
    x = np.asarray(x, dtype=np.float32)
    fc1_w = np.asarray(fc1_w, dtype=np.float32)
    fc1_b = np.asarray(fc1_b, dtype=np.float32)
    theta = np.asarray(theta_quantum, dtype=np.float32)
    fc_out_w = np.asarray(fc_out_w, dtype=np.float32)
    fc_out_b = np.asarray(fc_out_b, dtype=np.float32)

    w1t = np.ascontiguousarray(fc1_w.T).astype(bf16)          # [F, H]
    sin_theta = np.sin(theta)                                  # fp32
    w2t = np.ascontiguousarray(fc_out_w.T) * sin_theta[:, None]  # [H, C] fp32
    w2t = np.ascontiguousarray(w2t, dtype=np.float32)
    b2 = np.ascontiguousarray(fc_out_b.reshape(C, 1))

    xbf = x.astype(bf16)
    in_maps = []
    for i in range(NCORES):
        xs = xbf[i * R:(i + 1) * R]                            # [R, F]
        in_maps.append({
            "xt": np.ascontiguousarray(xs.T),                  # [F, R]
            "w1t": w1t,
            "b1": fc1_b,
            "w2t": w2t,
            "b2": b2,
        })
    return in_maps


def run(inputs, trace=False):
    """Run the bass kernel. Returns (logits [B, C] fp32, BassKernelResults)."""
    from concourse.bass_utils import run_bass_kernel_spmd

    nc = _get_program()
    in_maps = _prepare_in_maps(**inputs)
    res = run_bass_kernel_spmd(nc, in_maps, list(range(NCORES)), trace=trace)
    outT = np.concatenate([np.asarray(r["outT"]) for r in res.results], axis=1)
    logits = np.ascontiguousarray(outT.T, dtype=np.float32)    # [B, C]
    return logits, res


def kernel(**inputs) -> np.ndarray:
    logits, _ = run(inputs, trace=False)
    return logits


# revision 6
# speedup vs baseline: 19788.4611x; 19788.4611x over previous
"""Trainium2 Bass kernel for nn_BQNNModel (binary-quantum NN forward).

Reference computation (all fp32):
    h      = x @ fc1_w.T + fc1_b          # [B, H]
    h01    = clip((sign(h)+1)/2, 0, 1)    # {0, 0.5, 1}
    angle  = pi/2 + 0.5*(h01-0.5)*pi      # {pi/4, pi/2, 3pi/4}
    exp    = sin(angle) * sin(theta)[None]
    logits = exp @ fc_out_w.T + fc_out_b  # [B, C]

Sharding: pure data parallelism over batch across 8 cores (2048 rows each),
weights replicated.  No collectives needed (forward only).

Per-core device pipeline (hT layout — h is produced transposed so no on-chip
transposes are ever needed):
    xT  [1024, 2048] bf16 (host pre-cast + pre-transposed)
    hT[q-block] = sum_k W1T[k,q].T @ xT[k]        (TensorE, bf16, PSUM fp32)
    g   = (hT + b1[q]) > 0                        (VectorE, one tensor_scalar)
    sT  = Sin(g * (pi/2) + pi/4)                  (ScalarE activation, fp32)
    logitsT = sum_q W2T'[q].T @ sT[q]             (TensorE, fp32)
    outT = logitsT + b2                           (ScalarE Identity+bias)
where W2T'[q, c] = sin(theta_q) * fc_out_w[c, q] is folded on the host.

bf16 in matmul1 is safe: the output depends on h only through sign(h), and
sin(pi/4) == sin(3*pi/4), so even a sign flip of an |h|~0 element does not
change the result.
"""

import numpy as np
import ml_dtypes
from contextlib import ExitStack

B, F, H, C = 16384, 1024, 512, 10
NCORES = 8
R = B // NCORES          # 2048 rows per core
RC = 512                 # row chunk (matmul free dim)
P = 128
KB = F // P              # 8 contraction blocks
QB = H // P              # 4 hidden blocks
NCH = R // RC            # 4 row chunks per core

PI32 = np.float32(np.pi)
SIN_SCALE = float(np.float32(PI32 / np.float32(2.0)))   # pi/2 in fp32
SIN_BIAS = float(np.float32(PI32 / np.float32(4.0)))    # pi/4 in fp32

_CACHE = {}


def _build_program(loop_iters=0):
    import concourse.bass as bass  # noqa: F401
    import concourse.tile as tile
    from concourse import bacc, mybir

    nc = bacc.Bacc("TRN2", target_bir_lowering=False, debug=False,
                   num_devices=NCORES)

    xt = nc.dram_tensor("xt", [F, R], mybir.dt.bfloat16,
                        kind="ExternalInput").ap()
    w1t = nc.dram_tensor("w1t", [F, H], mybir.dt.bfloat16,
                         kind="ExternalInput").ap()
    b1 = nc.dram_tensor("b1", [H], mybir.dt.float32,
                        kind="ExternalInput").ap()
    w2t = nc.dram_tensor("w2t", [H, C], mybir.dt.float32,
                         kind="ExternalInput").ap()
    b2 = nc.dram_tensor("b2", [C, 1], mybir.dt.float32,
                        kind="ExternalInput").ap()
    outT = nc.dram_tensor("outT", [C, R], mybir.dt.float32,
                          kind="ExternalOutput").ap()

    with tile.TileContext(nc) as tc, ExitStack() as ctx:
        if loop_iters:
            with tc.For_i(0, loop_iters, 1,
                          hint_engines=(mybir.EngineType.PE,)):
                _kernel_body(ctx, tc, outT, xt, w1t, b1, w2t, b2, mybir)
        else:
            _kernel_body(ctx, tc, outT, xt, w1t, b1, w2t, b2, mybir)

    nc.compile()
    return nc


def _kernel_body(ctx, tc, outT, xt, w1t, b1, w2t, b2, mybir):
    nc = tc.nc

    consts = ctx.enter_context(tc.tile_pool(name="consts", bufs=1))
    xpool = ctx.enter_context(tc.tile_pool(name="xpool", bufs=2))
    gpool = ctx.enter_context(tc.tile_pool(name="gpool", bufs=3))
    spool = ctx.enter_context(tc.tile_pool(name="spool", bufs=2 * QB))
    opool = ctx.enter_context(tc.tile_pool(name="opool", bufs=1))
    psum1 = ctx.enter_context(tc.tile_pool(name="psum1", bufs=3, space="PSUM"))
    psum2 = ctx.enter_context(tc.tile_pool(name="psum2", bufs=2, space="PSUM"))

    xt_r = xt.rearrange("(ko p) r -> p ko r", p=P)

    # Replicated weights / biases.
    w1t_sb = consts.tile([P, KB, H], mybir.dt.bfloat16)
    nc.sync.dma_start(w1t_sb[:], w1t.rearrange("(ko p) h -> p ko h", p=P))
    w2t_sb = consts.tile([P, QB, C], mybir.dt.float32)
    nc.sync.dma_start(w2t_sb[:], w2t.rearrange("(qo p) c -> p qo c", p=P))
    b1_sb = consts.tile([P, QB], mybir.dt.float32)
    nc.sync.dma_start(b1_sb[:], b1.rearrange("(qo p) -> p qo", p=P))
    b2_sb = consts.tile([C, 1], mybir.dt.float32)
    nc.sync.dma_start(b2_sb[:], b2[:])
    sinb_sb = consts.tile([P, 1], mybir.dt.float32)
    nc.any.memset(sinb_sb[:], SIN_BIAS)

    out_sb = opool.tile([C, R], mybir.dt.float32)

    for c in range(NCH):
        x_sb = xpool.tile([P, KB, RC], mybir.dt.bfloat16)
        nc.sync.dma_start(x_sb[:], xt_r[:, :, c * RC:(c + 1) * RC])

        s_tiles = []
        for q in range(QB):
            hps = psum1.tile([P, RC], mybir.dt.float32)
            for k in range(KB):
                nc.tensor.matmul(
                    hps[:],
                    w1t_sb[:, k, q * P:(q + 1) * P],
                    x_sb[:, k, :],
                    start=(k == 0),
                    stop=(k == KB - 1),
                )
            # g = (h + b1) > 0  in {0.0, 1.0}; bf16 is exact for those.
            g = gpool.tile([P, RC], mybir.dt.bfloat16)
            nc.vector.tensor_scalar(
                out=g[:],
                in0=hps[:],
                scalar1=b1_sb[:, q:q + 1],
                scalar2=0.0,
                op0=mybir.AluOpType.add,
                op1=mybir.AluOpType.is_gt,
            )
            # sT = sin(g*pi/2 + pi/4)
            s = spool.tile([P, RC], mybir.dt.float32)
            nc.scalar.activation(
                s[:], g[:], mybir.ActivationFunctionType.Sin,
                bias=sinb_sb[:], scale=SIN_SCALE,
            )
            s_tiles.append(s)

        lps = psum2.tile([C, RC], mybir.dt.float32)
        for q in range(QB):
            nc.tensor.matmul(
                lps[:],
                w2t_sb[:, q, :],
                s_tiles[q][:],
                start=(q == 0),
                stop=(q == QB - 1),
            )
        # out = logits + b2 (per-partition bias), PSUM -> SBUF
        nc.scalar.activation(
            out_sb[:, c * RC:(c + 1) * RC], lps[:],
            mybir.ActivationFunctionType.Identity,
            bias=b2_sb[:], scale=1.0,
        )

    nc.sync.dma_start(outT[:], out_sb[:])


def _get_program(loop_iters=0):
    key = ("nc", loop_iters)
    if key not in _CACHE:
        _CACHE[key] = _build_program(loop_iters)
    return _CACHE[key]


def _prepare_in_maps(x, fc1_w, fc1_b, theta_quantum, fc_out_w, fc_out_b):
    bf16 = ml_dtypes.bfloat16
    x = np.asarray(x, dtype=np.float32)
    fc1_w = np.asarray(fc1_w, dtype=np.float32)
    fc1_b = np.asarray(fc1_b, dtype=np.float32)
    theta = np.asarray(theta_quantum, dtype=np.float32)
    fc_out_w = np.asarray(fc_out_w, dtype=np.float32)
    fc_out_b = np.asarray(fc_out_b, dtype=np.float32)

    w1t = np.ascontiguousarray(fc1_w.T).astype(bf16)          # [F, H]
    sin_theta = np.sin(theta)                                  # fp32
    w2t = np.ascontiguousarray(fc_out_w.T) * sin_theta[:, None]  # [H, C] fp32
    w2t = np.ascontiguousarray(w2t, dtype=np.float32)
    b2 = np.ascontiguousarray(fc_out_b.reshape(C, 1))

    xbf = x.astype(bf16)
    in_maps = []
    for i in range(NCORES):
        xs = xbf[i * R:(i + 1) * R]                            # [R, F]
        in_maps.append({
            "xt": np.ascontiguousarray(xs.T),                  # [F, R]
            "w1t": w1t,
            "b1": fc1_b,
            "w2t": w2t,
            "b2": b2,
        })
    return in_maps


def run(inputs, trace=False, loop_iters=0):
    """Run the bass kernel. Returns (logits [B, C] fp32, BassKernelResults)."""
    from concourse.bass_utils import run_bass_kernel_spmd

    nc = _get_program(loop_iters)
    in_maps = _prepare_in_maps(**inputs)
    res = run_bass_kernel_spmd(nc, in_maps, list(range(NCORES)), trace=trace)
    outT = np.concatenate([np.asarray(r["outT"]) for r in res.results], axis=1)
    logits = np.ascontiguousarray(outT.T, dtype=np.float32)    # [B, C]
    return logits, res


def kernel(**inputs) -> np.ndarray:
    logits, _ = run(inputs, trace=False)
    return logits


# revision 45
# speedup vs baseline: 36569.2344x; 1.8480x over previous
"""Trainium2 Bass kernel for nn_BQNNModel (binary-quantum NN forward).

Reference computation (all fp32):
    h      = x @ fc1_w.T + fc1_b          # [B, H]
    h01    = clip((sign(h)+1)/2, 0, 1)    # {0, 0.5, 1}
    angle  = pi/2 + 0.5*(h01-0.5)*pi      # {pi/4, pi/2, 3pi/4}
    exp    = sin(angle) * sin(theta)[None]
    logits = exp @ fc_out_w.T + fc_out_b  # [B, C]

Sharding: pure data parallelism over batch across 8 cores (2048 rows each),
weights replicated.  No collectives needed (forward only).

Per-core device pipeline (hT layout — h is produced transposed so no on-chip
transposes are ever needed):
    xT  [1024, 2048] (host pre-cast + pre-transposed)
    hT[q-block] = sum_k W1T[k,q].T @ xT[k]        (TensorE, PSUM fp32)
    g   = (hT + b1[q]) > 0                        (VectorE, one tensor_scalar)
    sT  = g*(sin(3pi/4) - sin(pi/4)) + sin(pi/4)  (VectorE; == sin(angle))
    logitsT = sum_q W2T'[q].T @ sT[q]             (TensorE)
    outT = logitsT + b2                           (bias add)
where W2T'[q, c] = sin(theta_q) * fc_out_w[c, q] is folded on the host and
the sin constants are the exact fp32 values the reference produces.

Reduced precision in matmul1 (bf16/fp8) is safe: the output depends on h
only through sign(h), and sin(pi/4) == sin(3*pi/4) bitwise in fp32, so a
sign flip of an |h|~0 element does not change the result.
"""

import numpy as np
import ml_dtypes
from contextlib import ExitStack

B, F, H, C = 16384, 1024, 512, 10
NCORES = 8
R = B // NCORES          # 2048 rows per core
RC = 512                 # row chunk (matmul free dim)
P = 128
KB = F // P              # 8 contraction blocks
QB = H // P              # 4 hidden blocks
NCH = R // RC            # 4 row chunks per core

PI32 = np.float32(np.pi)
SIN_SCALE = float(np.float32(PI32 / np.float32(2.0)))   # pi/2 in fp32
SIN_BIAS = float(np.float32(PI32 / np.float32(4.0)))    # pi/4 in fp32
# Exact fp32 constants the reference pipeline produces for the two branches.
C_NEG = float(np.sin(np.float32(PI32 / np.float32(4.0)), dtype=np.float32))
C_POS = float(np.sin(
    np.float32(PI32 / np.float32(2.0)) + np.float32(PI32 / np.float32(4.0)),
    dtype=np.float32))

# With the rescale trick, W2'' = C_NEG * W2T' is folded on the host and the
# on-chip select computes s' = g*(C_POS/C_NEG - 1) + 1, whose two values
# {1.0, C_POS/C_NEG} are exactly representable (C_POS == C_NEG in fp32, so
# both are exactly 1.0) — no quantization error on the matmul2 moving operand.
C_RATIO_M1 = float(np.float32(np.float32(C_POS) / np.float32(C_NEG))
                   - np.float32(1.0))

# ---- variant knobs (current best configuration) ----
# f8: fp8e4m3 + DoubleRow matmul1 (sign-safe), f32r matmul2 with host-side
# 11-bit pre-rounding + bias compensation (exact), binarize via ScalarE Sign,
# affine select + bias on VectorE.
MM1_DTYPE = "f8"        # "bf16" | "f8" (fp8e4m3 + DoubleRow)
MM2_MODE = "f32r"       # "f32" | "f32r" | "bf16"
S_MODE = "split"        # "act" | "dve" | "gps" | "mix" | "split"
BIN_ACT_TILES = 16      # for "split": how many of the 16 tiles binarize on ACT
AFF_ENG = "dve"         # for "split": engine for the affine pass (dve|gps)
W2_BLOB = False         # pack w2t+b1 into one host-laid-out DMA blob
DEBUG_G = False         # extra output with the binarized activations

_CACHE = {}


def _np_mm1_dtype():
    return ml_dtypes.float8_e4m3fn if MM1_DTYPE == "f8" else ml_dtypes.bfloat16


def _build_program(loop_iters=0):
    import concourse.bass as bass  # noqa: F401
    import concourse.tile as tile
    from concourse import bacc, mybir

    mm1_dt = (mybir.dt.float8e4 if MM1_DTYPE == "f8" else mybir.dt.bfloat16)

    nc = bacc.Bacc("TRN2", target_bir_lowering=False, debug=False,
                   num_devices=NCORES)

    xt = nc.dram_tensor("xt", [F, R], mm1_dt, kind="ExternalInput").ap()
    w1t = nc.dram_tensor("w1t", [F, H], mm1_dt, kind="ExternalInput").ap()
    b1 = nc.dram_tensor("b1", [H], mybir.dt.float32,
                        kind="ExternalInput").ap()
    if MM2_MODE == "bf16" and W2_BLOB:
        w2t = nc.dram_tensor("w2t", [P, 2 * (H // P) * C + (H // P)],
                             mybir.dt.bfloat16, kind="ExternalInput").ap()
    elif MM2_MODE == "bf16":
        w2t = nc.dram_tensor("w2t", [2 * H, C], mybir.dt.bfloat16,
                             kind="ExternalInput").ap()
    elif MM2_MODE == "f32r":
        w2t = nc.dram_tensor("w2t", [H, C], mybir.dt.float32r,
                             kind="ExternalInput").ap()
    else:
        w2t = nc.dram_tensor("w2t", [H, C], mybir.dt.float32,
                             kind="ExternalInput").ap()
    b2 = nc.dram_tensor("b2", [C, 1], mybir.dt.float32,
                        kind="ExternalInput").ap()
    outT = nc.dram_tensor("outT", [C, R], mybir.dt.float32,
                          kind="ExternalOutput").ap()
    outG = None
    if DEBUG_G:
        outG = nc.dram_tensor("outG", [P, QB * NCH, RC], mybir.dt.bfloat16,
                              kind="ExternalOutput").ap()

    with tile.TileContext(nc) as tc, ExitStack() as ctx:
        if loop_iters:
            with tc.For_i(0, loop_iters, 1,
                          hint_engines=(mybir.EngineType.PE,)):
                _kernel_body(ctx, tc, outT, xt, w1t, b1, w2t, b2, mybir, outG)
        else:
            _kernel_body(ctx, tc, outT, xt, w1t, b1, w2t, b2, mybir, outG)

    nc.compile()
    return nc


def _kernel_body(ctx, tc, outT, xt, w1t, b1, w2t, b2, mybir, outG=None):
    nc = tc.nc
    fp8_dr = MM1_DTYPE == "f8"
    mm1_dt = (mybir.dt.float8e4 if fp8_dr else mybir.dt.bfloat16)

    consts = ctx.enter_context(tc.tile_pool(name="consts", bufs=1))
    xpool = ctx.enter_context(tc.tile_pool(name="xpool", bufs=2))
    gpool = ctx.enter_context(tc.tile_pool(name="gpool", bufs=3))
    spool = ctx.enter_context(tc.tile_pool(name="spool", bufs=2 * QB))
    opool = ctx.enter_context(tc.tile_pool(name="opool", bufs=2))
    psum1 = ctx.enter_context(tc.tile_pool(name="psum1", bufs=3, space="PSUM"))
    psum2 = ctx.enter_context(tc.tile_pool(name="psum2", bufs=2, space="PSUM"))

    xt_r = xt.rearrange("(ko p) r -> p ko r", p=P)

    # Replicated weights / biases.  w1t split into independent half-tiles so
    # the first matmuls only wait for the first half (deps are per tile).
    w1t_r = w1t.rearrange("(ko p) h -> p ko h", p=P)
    KH = KB // 2
    w1t_lo = consts.tile([P, KH, H], mm1_dt)
    nc.sync.dma_start(w1t_lo[:], w1t_r[:, :KH, :])
    w1t_hi = consts.tile([P, KH, H], mm1_dt)
    nc.sync.dma_start(w1t_hi[:], w1t_r[:, KH:, :])

    def w1_slice(k, kspan, qsl):
        # k .. k+kspan-1 never crosses the half boundary (KH is even)
        t, off = (w1t_lo, 0) if k < KH else (w1t_hi, KH)
        return t[:, k - off:k - off + kspan, qsl]

    if MM2_MODE == "bf16" and W2_BLOB:
        # One host-packed blob in final SBUF layout: [w2 hi | w2 lo | b1],
        # all bf16 (b1 only feeds the sign compare, so bf16 bias is safe).
        wblob = consts.tile([P, 2 * QB * C + QB], mybir.dt.bfloat16)
        nc.sync.dma_start(wblob[:], w2t[:])

        def w2_slice(i):
            return wblob[:, i * C:(i + 1) * C]
        b1_sb = wblob[:, 2 * QB * C:]
    elif MM2_MODE == "bf16":
        w2t_sb = consts.tile([P, 2 * QB, C], mybir.dt.bfloat16)
        nc.sync.dma_start(
            w2t_sb[:], w2t.rearrange("(s qo p) c -> p (s qo) c", p=P, s=2))

        def w2_slice(i):
            return w2t_sb[:, i, :]
        b1_sb = consts.tile([P, QB], mybir.dt.float32)
        nc.sync.dma_start(b1_sb[:], b1.rearrange("(qo p) -> p qo", p=P))
    else:
        w2t_dt = (mybir.dt.float32r if MM2_MODE == "f32r"
                  else mybir.dt.float32)
        w2t_sb = consts.tile([P, QB, C], w2t_dt)
        nc.sync.dma_start(w2t_sb[:],
                          w2t.rearrange("(qo p) c -> p qo c", p=P))

        def w2_slice(i):
            return w2t_sb[:, i, :]
        b1_sb = consts.tile([P, QB], mybir.dt.float32)
        nc.sync.dma_start(b1_sb[:], b1.rearrange("(qo p) -> p qo", p=P))
    b2_sb = consts.tile([C, 1], mybir.dt.float32)
    nc.sync.dma_start(b2_sb[:], b2[:])

    s_np_dt = {"bf16": mybir.dt.bfloat16,
               "f32r": mybir.dt.float32r,
               "f32": mybir.dt.float32}[MM2_MODE]

    def emit_mm2(c, s_tiles):
        lps = psum2.tile([C, RC], mybir.dt.float32)
        if MM2_MODE == "bf16":
            for i, (part, qq) in enumerate(
                    [(part, q) for part in range(2) for q in range(QB)]):
                nc.tensor.matmul(
                    lps[:],
                    w2_slice(part * QB + qq),
                    s_tiles[qq][:],
                    start=(i == 0),
                    stop=(i == 2 * QB - 1),
                )
        else:
            for q in range(QB):
                nc.tensor.matmul(
                    lps[:], w2_slice(q), s_tiles[q][:],
                    start=(q == 0), stop=(q == QB - 1),
                )
        # out = logits + b2 (per-partition bias), PSUM -> SBUF
        out_sb = opool.tile([C, RC], mybir.dt.float32)
        nc.vector.tensor_scalar(
            out=out_sb[:], in0=lps[:],
            scalar1=b2_sb[:], scalar2=None,
            op0=mybir.AluOpType.add,
        )
        # stream this chunk's output out right away
        nc.sync.dma_start(outT[:, c * RC:(c + 1) * RC], out_sb[:])

    prev_s = None
    for c in range(NCH):
        csl = slice(c * RC, (c + 1) * RC)
        if c == 0:
            # independent half-tiles so the first matmuls only wait on x_lo
            x_lo = xpool.tile([P, KH, RC], mm1_dt, tag="x_lo")
            nc.sync.dma_start(x_lo[:], xt_r[:, :KH, csl])
            x_hi = xpool.tile([P, KH, RC], mm1_dt, tag="x_hi")
            nc.sync.dma_start(x_hi[:], xt_r[:, KH:, csl])

            def x_slice(k, kspan, x_lo=x_lo, x_hi=x_hi):
                t, off = (x_lo, 0) if k < KH else (x_hi, KH)
                return t[:, k - off:k - off + kspan, :]
        else:
            x_sb = xpool.tile([P, KB, RC], mm1_dt, tag="x_full")
            nc.sync.dma_start(x_sb[:], xt_r[:, :, csl])

            def x_slice(k, kspan, x_sb=x_sb):
                return x_sb[:, k:k + kspan, :]

        s_tiles = []
        for q in range(QB):
            hps = psum1.tile([P, RC], mybir.dt.float32)
            if fp8_dr:
                for k in range(0, KB, 2):
                    nc.tensor.matmul(
                        hps[:],
                        w1_slice(k, 2, slice(q * P, (q + 1) * P)),
                        x_slice(k, 2),
                        start=(k == 0),
                        stop=(k == KB - 2),
                        perf_mode=mybir.MatmulPerfMode.DoubleRow,
                    )
            else:
                for k in range(KB):
                    nc.tensor.matmul(
                        hps[:],
                        w1_slice(k, 1, slice(q * P, (q + 1) * P)),
                        x_slice(k, 1),
                        start=(k == 0),
                        stop=(k == KB - 1),
                    )
            s = spool.tile([P, RC], s_np_dt)
            tile_idx = c * QB + q
            if S_MODE == "split":
                aff = nc.gpsimd if AFF_ENG == "gps" else nc.vector
                if tile_idx % 16 < (BIN_ACT_TILES % 17):
                    # t = Sign(h + b1) in {-1, 1} on ScalarE, then
                    # s' = t*(r-1)/2 + (r+1)/2
                    t = gpool.tile([P, RC], mybir.dt.bfloat16)
                    nc.scalar.activation(
                        t[:], hps[:], mybir.ActivationFunctionType.Sign,
                        bias=b1_sb[:, q:q + 1], scale=1.0,
                    )
                    g = t
                    aff.tensor_scalar(
                        out=s[:], in0=t[:],
                        scalar1=C_RATIO_M1 / 2.0,
                        scalar2=float(np.float32(C_RATIO_M1 / 2.0) + 1.0),
                        op0=mybir.AluOpType.mult, op1=mybir.AluOpType.add,
                    )
                else:
                    # g = (h + b1) > 0 on DVE, then s' = g*(r-1) + 1
                    g = gpool.tile([P, RC], mybir.dt.bfloat16)
                    nc.vector.tensor_scalar(
                        out=g[:], in0=hps[:],
                        scalar1=b1_sb[:, q:q + 1], scalar2=0.0,
                        op0=mybir.AluOpType.add, op1=mybir.AluOpType.is_gt,
                    )
                    aff.tensor_scalar(
                        out=s[:], in0=g[:],
                        scalar1=C_RATIO_M1, scalar2=1.0,
                        op0=mybir.AluOpType.mult, op1=mybir.AluOpType.add,
                    )
            else:
                # g = (h + b1) > 0 in {0.0, 1.0}
                g = gpool.tile([P, RC], mybir.dt.bfloat16)
                nc.vector.tensor_scalar(
                    out=g[:], in0=hps[:],
                    scalar1=b1_sb[:, q:q + 1], scalar2=0.0,
                    op0=mybir.AluOpType.add, op1=mybir.AluOpType.is_gt,
                )
                # s' = g*(C_POS/C_NEG - 1) + 1  (== sin(angle)/C_NEG)
                eng = S_MODE
                if S_MODE == "mix":
                    eng = ("act", "gps")[q % 2]
                if eng == "act":
                    nc.scalar.activation(
                        s[:], g[:], mybir.ActivationFunctionType.Copy,
                        bias=1.0, scale=C_RATIO_M1,
                    )
                else:
                    veng = nc.gpsimd if eng == "gps" else nc.vector
                    veng.tensor_scalar(
                        out=s[:], in0=g[:],
                        scalar1=C_RATIO_M1, scalar2=1.0,
                        op0=mybir.AluOpType.mult, op1=mybir.AluOpType.add,
                    )
            if outG is not None:
                nc.sync.dma_start(outG[:, c * QB + q, :], g[:])
            s_tiles.append(s)

        # Software pipelining: emit the previous chunk's mm2 AFTER this
        # chunk's mm1 block so PE's in-order queue never stalls on the
        # elementwise chain.
        if prev_s is not None:
            emit_mm2(c - 1, prev_s)
        prev_s = s_tiles

    emit_mm2(NCH - 1, prev_s)


def _get_program(loop_iters=0):
    key = ("nc", loop_iters, MM1_DTYPE, MM2_MODE, S_MODE, BIN_ACT_TILES,
           AFF_ENG, W2_BLOB, DEBUG_G)
    if key not in _CACHE:
        _CACHE[key] = _build_program(loop_iters)
    return _CACHE[key]


def _prepare_in_maps(x, fc1_w, fc1_b, theta_quantum, fc_out_w, fc_out_b):
    x = np.asarray(x, dtype=np.float32)
    fc1_w = np.asarray(fc1_w, dtype=np.float32)
    fc1_b = np.asarray(fc1_b, dtype=np.float32)
    theta = np.asarray(theta_quantum, dtype=np.float32)
    fc_out_w = np.asarray(fc_out_w, dtype=np.float32)
    fc_out_b = np.asarray(fc_out_b, dtype=np.float32)

    mm1_np = _np_mm1_dtype()
    w1t = np.ascontiguousarray(fc1_w.T).astype(mm1_np)         # [F, H]
    sin_theta = np.sin(theta)                                  # fp32
    w2t = np.ascontiguousarray(fc_out_w.T) * sin_theta[:, None]  # [H, C] fp32
    w2t = w2t * np.float32(C_NEG)            # rescale trick: s' = s / C_NEG
    w2t = np.ascontiguousarray(w2t, dtype=np.float32)
    if MM2_MODE == "f32r":
        # The PE reads float32r operands rounded to 11 mantissa bits (RNE,
        # probed on hardware).  Pre-round W2 on the host so the device sees
        # exactly these values, and fold the rounding residual into b2 —
        # exact because the moving operand s' is identically 1.0.
        u = w2t.view(np.uint32).astype(np.uint64)
        rnd = ((u + (1 << 11) - 1 + ((u >> 12) & 1)) >> 12 << 12)
        w2r = rnd.astype(np.uint32).view(np.float32)
        delta = (w2t.astype(np.float64) - w2r.astype(np.float64)).sum(axis=0)
        fc_out_b = (fc_out_b.astype(np.float64) + delta).astype(np.float32)
        w2t = np.ascontiguousarray(w2r)
    if MM2_MODE == "bf16":
        bf16 = ml_dtypes.bfloat16
        hi = w2t.astype(bf16)
        lo = (w2t - hi.astype(np.float32)).astype(bf16)
        if W2_BLOB:
            # blob[p,(s*QB+q)*C+c] = part_s[q*P+p, c]; blob[p, 2QB*C+q] = b1
            wb = np.zeros((P, 2 * QB * C + QB), dtype=bf16)
            for s_i, part in enumerate((hi, lo)):
                r = (part.reshape(QB, P, C).transpose(1, 0, 2)
                     .reshape(P, QB * C))
                wb[:, s_i * QB * C:(s_i + 1) * QB * C] = r
            wb[:, 2 * QB * C:] = fc1_b.reshape(QB, P).T.astype(bf16)
            w2t_send = np.ascontiguousarray(wb)
        else:
            w2t_send = np.ascontiguousarray(
                np.stack([hi, lo], axis=0).reshape(2 * H, C))
    else:
        w2t_send = w2t
    b2 = np.ascontiguousarray(fc_out_b.reshape(C, 1))

    xq = x.astype(mm1_np)
    in_maps = []
    for i in range(NCORES):
        xs = xq[i * R:(i + 1) * R]                             # [R, F]
        in_maps.append({
            "xt": np.ascontiguousarray(xs.T),                  # [F, R]
            "w1t": w1t,
            "b1": fc1_b,
            "w2t": w2t_send,
            "b2": b2,
        })
    return in_maps


def run(inputs, trace=False, loop_iters=0):
    """Run the bass kernel. Returns (logits [B, C] fp32, BassKernelResults)."""
    from concourse.bass_utils import run_bass_kernel_spmd

    nc = _get_program(loop_iters)
    in_maps = _prepare_in_maps(**inputs)
    res = run_bass_kernel_spmd(nc, in_maps, list(range(NCORES)), trace=trace)
    outT = np.concatenate([np.asarray(r["outT"]) for r in res.results], axis=1)
    logits = np.ascontiguousarray(outT.T, dtype=np.float32)    # [B, C]
    return logits, res


def kernel(**inputs) -> np.ndarray:
    logits, _ = run(inputs, trace=False)
    return logits
